# revision 11
# baseline (speedup 1.0000x reference)
"""Differential attention kernel for Trainium2, 8 NeuronCores.

Sharding: B(2) x head-groups(4) -> 8 cores; each core computes 3 heads'
differential attention for one batch element plus its partial slice of the
output projection (row-parallel over Wo). Host sums the 4 partials per batch
element and adds bo.

v2 pipeline (per core, all matmuls bf16, fp32 PSUM accum):
  - scores for the two branches run CONCURRENTLY in the PE via row tiling
    (tile_position (0,0) and (64,0)): branch-b q^T/k^T live on SBUF
    partitions b*64..b*64+64, each score matmul contracts over its 64-row
    group, both stream at once -> 2x score throughput.
  - exp batched: ONE ScalarE activation per (head, q-block, strip) covering
    both branches' S^T tiles ([128, 2, 512]) -> fewer, larger ACT calls.
  - PV keeps the [v|1] M=65 trick (denominator rides as psum row 64),
    software-pipelined one strip behind exp.
  - next head's q/k projections are drip-fed into the attention strip loop
    so the PE's slack under the ACT-bound steady state does the projections
    for free; ACT starts on head 0 ~40us earlier than a serial-proj design.
  - output projection contracts 192 rows as 128+64 packed matmuls.
"""

import os
import sys
from contextlib import ExitStack

for _p in ("/opt/trn_rl_repo", "/root/.axon_site/_ro/trn_rl_repo"):
    if os.path.isdir(_p) and _p not in sys.path:
        sys.path.insert(0, _p)

import ml_dtypes
import numpy as np

import concourse.bass as bass
import concourse.bacc as bacc_mod
import concourse.mybir as mybir
from concourse.bass_utils import run_bass_kernel_spmd
from concourse.tile import TileContext

BF16 = ml_dtypes.bfloat16
F = mybir.dt

B, N, C, H, D = 2, 2048, 768, 12, 64
HPC = 3          # heads per core
NCORES = 8
NT = N // 128    # 16 key strips
QB = 512         # query-block width (one PSUM bank of fp32)
NQ = N // QB     # 4 query blocks



def _proj_steps(nc, tc, pjp, xt_sb, wqk_sb, qk_sb, h):
    """Generator of q/k projection steps for head h: 8 groups (q,k x 4
    query-quarters), each 6 accumulating matmuls + 1 psum->SBUF copy."""
    fp32 = F.float32
    for t in range(2):          # 0 = q, 1 = k
        blk = (2 * h + t) * 128
        for g in range(4):
            pp = pjp.tile([128, 512], fp32, tag="pj", name="pp")
            for c in range(6):
                nc.tensor.matmul(
                    pp,
                    lhsT=wqk_sb[:, c, blk : blk + 128],
                    rhs=xt_sb[:, c, g * 512 : (g + 1) * 512],
                    start=(c == 0),
                    stop=(c == 5),
                )
                yield
            nc.vector.tensor_copy(qk_sb[:, h, t, g * 512 : (g + 1) * 512], pp)
            yield


def _body(nc, tc, ctx, xt, wqk, wv, wo, lamc, out, taps=None):
    fp32, bf16 = F.float32, F.bfloat16
    Exp = mybir.ActivationFunctionType.Exp

    singles = ctx.enter_context(tc.tile_pool(name="singles", bufs=1))
    woA_sb = singles.tile([128, C], bf16)          # Wo rows for heads 0,1
    woB_sb = singles.tile([64, C], bf16)           # Wo rows for head 2
    lams_sb = singles.tile([128, 6], fp32)         # col u: 1.0 (br0) / -lam (br1)
    xt_sb = singles.tile([128, 6, N], bf16)        # x^T, c = ch*128+p
    wqk_sb = singles.tile([128, 6, C], bf16)       # blocks [q0 k0 q1 k1 q2 k2]
    wv_sb = singles.tile([128, 6, HPC * D], bf16)
    qk_sb = singles.tile([128, HPC, 2, N], bf16)   # [part(br*64+d), h, q/k, n]
    v_sb = singles.tile([128, NT, HPC, D + 1], bf16)
    u_sb = singles.tile([65, HPC, 2, N], fp32)     # rows 0:64 u, row 64 denom
    ddA = singles.tile([128, N], bf16)             # diff for heads 0,1
    ddB = singles.tile([64, N], bf16)              # diff for head 2
    r_dram = nc.dram_tensor("r_bounce", [6, N], bf16)

    nc.sync.dma_start(out=woA_sb, in_=wo[0:128, :])
    nc.sync.dma_start(out=woB_sb, in_=wo[128:192, :])
    nc.sync.dma_start(out=lams_sb, in_=lamc[:, :])
    xt_r = xt[:, :].rearrange("(ch p) n -> p ch n", p=128)
    wqk_r = wqk[:, :].rearrange("(ch p) w -> p ch w", p=128)
    wv_r = wv[:, :].rearrange("(ch p) w -> p ch w", p=128)
    for c in range(6):
        nc.sync.dma_start(out=wv_sb[:, c, :], in_=wv_r[:, c, :])
    for c in range(6):
        eng = nc.sync if c % 2 == 0 else nc.gpsimd
        eng.dma_start(out=xt_sb[:, c, :], in_=xt_r[:, c, :])
    for c in range(6):
        nc.sync.dma_start(out=wqk_sb[:, c, :], in_=wqk_r[:, c, :])
    nc.vector.memset(v_sb[:, :, :, D : D + 1], 1.0)

    # pre-warm PE clock gate + preload the exp table during the input DMA wait
    with tc.tile_pool(name="warm_sb", bufs=1) as warm_sb, \
         tc.tile_pool(name="warm_ps", bufs=1, space="PSUM") as warm_ps:
        wsrc = warm_sb.tile([128, 512], bf16)
        nc.vector.memset(wsrc, 0.0)
        wdst = warm_sb.tile([128, 16], bf16)
        wt = warm_ps.tile([128, 512], fp32)
        nc.scalar.activation(wdst, wsrc[:, 0:16], Exp)
        for _ in range(20):
            nc.tensor.matmul(wt, lhsT=wsrc[:, 0:128], rhs=wsrc, start=True, stop=True)

    # ---------- projections (own PSUM scope; must fully precede attention:
    # proj matmuls interleaved with the row-tiled scores come back corrupt) --
    with tc.tile_pool(name="pjp", bufs=2, space="PSUM") as pjp:
        # v projection, all heads (one strip per psum bank: a group's
        # start=True clears its whole bank, so groups never share a bank)
        for ti in range(NT):
            vp = pjp.tile([128, HPC * D], fp32, tag="pj", name="vp")
            for c in range(6):
                nc.tensor.matmul(
                    vp,
                    lhsT=xt_sb[:, c, ti * 128 : (ti + 1) * 128],
                    rhs=wv_sb[:, c, :],
                    start=(c == 0),
                    stop=(c == 5),
                )
            nc.vector.tensor_copy(
                v_sb[:, ti, :, 0:D], vp.rearrange("p (h d) -> p h d", h=HPC)
            )
        for hh in range(HPC):
            for _ in _proj_steps(nc, tc, pjp, xt_sb, wqk_sb, qk_sb, hh):
                pass

    # EXP group pattern per (h, qi): 2-strip "big" tiles alternating with
    # 1-strip "small" ones -> ACT calls of 2048/1024 elems, amortizing the
    # ~280-cycle per-instruction overhead while fitting 8 PSUM banks
    GROUPS = [(0, 1), (2,), (3, 4), (5,), (6, 7), (8,),
              (9, 10), (11,), (12, 13), (14,), (15,)]

    with tc.tile_pool(name="stB", bufs=1, space="PSUM") as stB, \
         tc.tile_pool(name="stS", bufs=1, space="PSUM") as stS, \
         tc.tile_pool(name="upp", bufs=1, space="PSUM") as upp, \
         tc.tile_pool(name="ptp", bufs=2) as ptp, \
         tc.tile_pool(name="rsc", bufs=3) as rsc:

        dens = {}

        def emit_rchain(h, qi):
            """reciprocal of the spread denominators for one (h, qi) slot;
            result lands in r_dram (bf16, -lam folded for branch 1)."""
            for br in range(2):
                u = 2 * h + br
                rq = rsc.tile([128, 4], fp32, tag=f"rq{br}", name=f"rq{br}")
                nc.vector.reciprocal(rq, dens[h][br][:, qi * 4 : qi * 4 + 4])
                rbq = rsc.tile([128, 4], bf16, tag=f"rBq{br}", name=f"rBq{br}")
                nc.vector.tensor_scalar_mul(rbq, rq, lams_sb[:, u : u + 1])
                nc.sync.dma_start(
                    out=r_dram[u : u + 1, qi * QB : (qi + 1) * QB], in_=rbq
                )

        def emit_diff(h, qi):
            """diff_h[:, qi] = u0*r0 - lam*u1*r1 (lam folded into r1)."""
            q0 = qi * QB
            rb = []
            for br in range(2):
                u = 2 * h + br
                rbx = rsc.tile([64, QB], bf16, tag=f"rb{br}", name=f"rb{br}")
                nc.sync.dma_start(
                    out=rbx,
                    in_=r_dram[u : u + 1, q0 : q0 + QB].partition_broadcast(64),
                )
                rb.append(rbx)
            t1 = rsc.tile([64, QB], bf16, tag="t1", name="t1")
            nc.vector.tensor_mul(t1, u_sb[0:64, h, 0, q0 : q0 + QB], rb[0])
            t2 = rsc.tile([64, QB], bf16, tag="t2", name="t2")
            nc.vector.tensor_mul(t2, u_sb[0:64, h, 1, q0 : q0 + QB], rb[1])
            dst = (ddA[h * 64 : (h + 1) * 64, q0 : q0 + QB] if h < 2
                   else ddB[:, q0 : q0 + QB])
            nc.vector.tensor_add(dst, t1, t2)

        # ---------- attention ----------
        slots = [(h, qi) for h in range(HPC) for qi in range(NQ)]
        for si, (h, qi) in enumerate(slots):
            if qi == 0:
                dens[h] = [
                    rsc.tile([128, NQ * 4], fp32, tag=f"den{br}", name=f"den{br}")
                    for br in range(2)
                ]
            q0 = qi * QB
            u_pair = [
                upp.tile([65, QB], fp32, tag=f"u{br}", name=f"u_ps{br}")
                for br in range(2)
            ]
            prev = []   # [(pt_tile, strip-slice index or None, strip)]
            for g in GROUPS:
                if len(g) == 2:
                    st = stB.tile([128, 2, 2, QB], fp32, tag="stB", name="stb")
                    pt = ptp.tile([128, 2, 2, QB], bf16, tag="ptB", name="ptb")
                else:
                    st = stS.tile([128, 2, QB], fp32, tag="stS", name="sts")
                    pt = ptp.tile([128, 2, QB], bf16, tag="ptS", name="pts")
                for j, s in enumerate(g):
                    stj = st[:, j, :, :] if len(g) == 2 else st
                    for br in range(2):
                        p0 = br * 64
                        nc.tensor.matmul(
                            stj[:, br, :],
                            lhsT=qk_sb[p0 : p0 + 64, h, 1, s * 128 : (s + 1) * 128],
                            rhs=qk_sb[p0 : p0 + 64, h, 0, q0 : q0 + QB],
                            start=True,
                            stop=True,
                            tile_position=(p0, 0),
                        )
                nc.scalar.activation(pt, st, Exp)
                for ptt, j, s in prev:
                    ptj = ptt[:, j, :, :] if j is not None else ptt
                    for br in range(2):
                        nc.tensor.matmul(
                            u_pair[br],
                            lhsT=v_sb[:, s, h, :],
                            rhs=ptj[:, br, :],
                            start=(s == 0),
                            stop=(s == NT - 1),
                        )
                prev = [(pt, (j if len(g) == 2 else None), s)
                        for j, s in enumerate(g)]
            for ptt, j, s in prev:
                ptj = ptt[:, j, :, :] if j is not None else ptt
                for br in range(2):
                    nc.tensor.matmul(
                        u_pair[br],
                        lhsT=v_sb[:, s, h, :],
                        rhs=ptj[:, br, :],
                        start=(s == 0),
                        stop=(s == NT - 1),
                    )
            for br in range(2):
                nc.vector.tensor_copy(u_sb[:, h, br, q0 : q0 + QB], u_pair[br])
                # spread denom [1,QB] -> [128, QB/128] for full-lane recip
                nc.sync.dma_start(
                    out=dens[h][br][:, qi * 4 : qi * 4 + 4],
                    in_=u_sb[64:65, h, br, q0 : q0 + QB],
                )
            # lagged post-processing so DMA latency hides under attention:
            # recip chain one slot behind, diff two slots behind
            if si >= 1:
                emit_rchain(*slots[si - 1])
            if si >= 2:
                emit_diff(*slots[si - 2])
        emit_rchain(*slots[-1])
        # PE keep-warm: gated on the last u evacuation via their operands,
        # these junk matmuls keep the HAM clock at 2.4 GHz through the final
        # recip/diff DMA chain so the output projection is not throttled
        junk = stS.tile([128, 2, QB], fp32, tag="stS", name="junk")
        for s in range(6):
            off = (NQ - 1) * QB + (s % 4) * 128
            nc.tensor.matmul(
                junk[:, 0, :],
                lhsT=u_sb[0:64, HPC - 1, 1, off : off + 128],
                rhs=u_sb[0:64, HPC - 1, 1, (NQ - 1) * QB : NQ * QB],
                start=True,
                stop=True,
            )
        emit_diff(*slots[-2])
        emit_diff(*slots[-1])
        if taps:
            nc.sync.dma_start(out=taps["qk"][:, :, :, :], in_=qk_sb)
            nc.sync.dma_start(out=taps["v"][:, :, :, :], in_=v_sb)
            nc.sync.dma_start(out=taps["u"][:, :, :, :], in_=u_sb)
            nc.sync.dma_start(out=taps["r"][0, :, :], in_=r_dram[:, :])
            nc.sync.dma_start(out=taps["ddA"][:, :], in_=ddA)
            nc.sync.dma_start(out=taps["ddB"][:, :], in_=ddB)

    # ---------- output projection: contraction 192 = 128 (h0,h1) + 64 (h2) ----
    with tc.tile_pool(name="fpp", bufs=3, space="PSUM") as fpp, \
         tc.tile_pool(name="outp", bufs=3) as outp:
        for ti in range(NT):
            fo = fpp.tile([128, C], fp32)
            for o, w in ((0, 512), (512, 256)):
                nc.tensor.matmul(
                    fo[:, o : o + w],
                    lhsT=ddA[:, ti * 128 : (ti + 1) * 128],
                    rhs=woA_sb[:, o : o + w],
                    start=True,
                    stop=False,
                )
                nc.tensor.matmul(
                    fo[:, o : o + w],
                    lhsT=ddB[:, ti * 128 : (ti + 1) * 128],
                    rhs=woB_sb[:, o : o + w],
                    start=False,
                    stop=True,
                )
            ot = outp.tile([128, C], bf16)
            nc.vector.tensor_copy(ot, fo)
            oeng = nc.sync if ti % 2 == 0 else nc.gpsimd
            oeng.dma_start(out=out[ti * 128 : (ti + 1) * 128, :], in_=ot)


def build_bass(debug_taps=False):
    nc = bacc_mod.Bacc(None)
    xt = nc.dram_tensor("xt", [C, N], F.bfloat16, kind="ExternalInput")
    wqk = nc.dram_tensor("wqk", [C, C], F.bfloat16, kind="ExternalInput")
    wv = nc.dram_tensor("wv", [C, HPC * D], F.bfloat16, kind="ExternalInput")
    wo = nc.dram_tensor("wo", [HPC * D, C], F.bfloat16, kind="ExternalInput")
    lamc = nc.dram_tensor("lamc", [128, 6], F.float32, kind="ExternalInput")
    out = nc.dram_tensor("out", [N, C], F.bfloat16, kind="ExternalOutput")
    taps = None
    if debug_taps:
        taps = {
            "qk": nc.dram_tensor("tap_qk", [128, HPC, 2, N], F.bfloat16, kind="ExternalOutput"),
            "v": nc.dram_tensor("tap_v", [128, NT, HPC, D + 1], F.bfloat16, kind="ExternalOutput"),
            "u": nc.dram_tensor("tap_u", [65, HPC, 2, N], F.float32, kind="ExternalOutput"),
            "r": nc.dram_tensor("tap_r", [1, 6, N], F.bfloat16, kind="ExternalOutput"),
            "ddA": nc.dram_tensor("tap_ddA", [128, N], F.bfloat16, kind="ExternalOutput"),
            "ddB": nc.dram_tensor("tap_ddB", [64, N], F.bfloat16, kind="ExternalOutput"),
        }
    with TileContext(nc) as tc:
        with ExitStack() as ctx:
            _body(nc, tc, ctx, xt, wqk, wv, wo, lamc, out, taps=taps)
    nc.compile()
    return nc


_NC = None


def _get_nc():
    global _NC
    if _NC is None:
        _NC = build_bass()
    return _NC


def _prep_core(core, x, Wq, Wk, Wv, Wo, lam):
    b = core // 4
    heads = [(core % 4) * HPC + i for i in range(HPC)]
    sc = 1.0 / np.sqrt(D)
    xt = np.ascontiguousarray(x[b].T).astype(BF16)
    # block layout [q_h0 | k_h0 | q_h1 | k_h1 | q_h2 | k_h2], within each
    # 128-col block: cols 0:64 = branch0, cols 64:128 = branch1
    wqk = np.empty((C, C), np.float32)
    for i, h in enumerate(heads):
        qb, kb = (2 * i) * 128, (2 * i + 1) * 128
        for br in range(2):
            wqk[:, qb + br * 64 : qb + br * 64 + 64] = (
                Wq[:, br * C + h * D : br * C + (h + 1) * D] * sc
            )
            wqk[:, kb + br * 64 : kb + br * 64 + 64] = (
                Wk[:, br * C + h * D : br * C + (h + 1) * D]
            )
    wv = np.concatenate([Wv[:, h * D : (h + 1) * D] for h in heads], axis=1)
    wo = np.concatenate([Wo[h * D : (h + 1) * D, :] for h in heads], axis=0)
    lams = np.zeros((128, 6), np.float32)
    for i, h in enumerate(heads):
        lams[:, 2 * i] = 1.0
        lams[:, 2 * i + 1] = -lam[h]
    return dict(
        xt=xt,
        wqk=wqk.astype(BF16),
        wv=wv.astype(BF16),
        wo=wo.astype(BF16),
        lamc=lams,
    )


def kernel(x, Wq, Wk, Wv, lambda_p, Wo, bo, _trace=False, _tmpdir=None):
    x = np.asarray(x, np.float32)
    lam = np.exp(np.asarray(lambda_p, np.float32).reshape(H))
    in_maps = [
        _prep_core(core, x, np.asarray(Wq, np.float32), np.asarray(Wk, np.float32),
                   np.asarray(Wv, np.float32), np.asarray(Wo, np.float32), lam)
        for core in range(NCORES)
    ]
    nc = _get_nc()
    res = run_bass_kernel_spmd(
        nc, in_maps, list(range(NCORES)), trace=_trace, tmpdir=_tmpdir
    )
    outf = np.zeros((B, N, C), np.float32)
    for core in range(NCORES):
        outf[core // 4] += np.asarray(res.results[core]["out"], np.float32)
    outf += np.asarray(bo, np.float32)[None, None, :]
    if _trace:
        kernel.last_exec_time_ns = res.exec_time_ns
    return outf


# revision 18
# speedup vs baseline: 1.0840x; 1.0840x over previous
"""Differential attention kernel for Trainium2, 8 NeuronCores.

Sharding: B(2) x head-groups(4) -> 8 cores; each core computes 3 heads'
differential attention for one batch element plus its partial slice of the
output projection (row-parallel over Wo). Host sums the 4 partials per batch
element and adds bo.

v2 pipeline (per core, all matmuls bf16, fp32 PSUM accum):
  - scores for the two branches run CONCURRENTLY in the PE via row tiling
    (tile_position (0,0) and (64,0)): branch-b q^T/k^T live on SBUF
    partitions b*64..b*64+64, each score matmul contracts over its 64-row
    group, both stream at once -> 2x score throughput.
  - exp batched: ONE ScalarE activation per (head, q-block, strip) covering
    both branches' S^T tiles ([128, 2, 512]) -> fewer, larger ACT calls.
  - PV keeps the [v|1] M=65 trick (denominator rides as psum row 64),
    software-pipelined one strip behind exp.
  - next head's q/k projections are drip-fed into the attention strip loop
    so the PE's slack under the ACT-bound steady state does the projections
    for free; ACT starts on head 0 ~40us earlier than a serial-proj design.
  - output projection contracts 192 rows as 128+64 packed matmuls.
"""

import os
import sys
from contextlib import ExitStack

for _p in ("/opt/trn_rl_repo", "/root/.axon_site/_ro/trn_rl_repo"):
    if os.path.isdir(_p) and _p not in sys.path:
        sys.path.insert(0, _p)

import ml_dtypes
import numpy as np

import concourse.bass as bass
import concourse.bacc as bacc_mod
import concourse.mybir as mybir
from concourse.bass_utils import run_bass_kernel_spmd
from concourse.tile import TileContext

BF16 = ml_dtypes.bfloat16
F = mybir.dt

B, N, C, H, D = 2, 2048, 768, 12, 64
HPC = 3          # heads per core
NCORES = 8
NT = N // 128    # 16 key strips
QB = 512         # query-block width (one PSUM bank of fp32)
NQ = N // QB     # 4 query blocks



def _proj_head(nc, pjp, pjt, xt_sb, wqk_sb, qk_sb, h):
    """q/k projection for head h: 8 groups (q,k x 4 query-quarters), each
    contraction-split onto PE row-groups T0/T8 (two concurrent 64-row
    matmul chains into separate psum banks); ScalarE evacuates the T8
    partial, DVE merges PSUM + SBUF into qk_sb."""
    fp32, bf16 = F.float32, F.bfloat16
    for t in range(2):          # 0 = q, 1 = k
        blk = (2 * h + t) * 128
        for g in range(4):
            ppA = pjp.tile([128, 512], fp32, tag="pjA", name="ppA")
            ppB = pjp.tile([128, 512], fp32, tag="pjB", name="ppB")
            for c in range(6):
                for p0, pp in ((0, ppA), (64, ppB)):
                    nc.tensor.matmul(
                        pp,
                        lhsT=wqk_sb[p0 : p0 + 64, c, blk : blk + 128],
                        rhs=xt_sb[p0 : p0 + 64, c, g * 512 : (g + 1) * 512],
                        start=(c == 0),
                        stop=(c == 5),
                        tile_position=(p0, 0),
                    )
            tpB = pjt.tile([128, 512], bf16, tag="tpB", name="tpB")
            nc.scalar.copy(tpB, ppB)
            nc.vector.tensor_add(
                qk_sb[:, h, t, g * 512 : (g + 1) * 512], ppA, tpB
            )


def _body(nc, tc, ctx, xt, wqk, wv, wo, lamc, out, taps=None):
    fp32, bf16 = F.float32, F.bfloat16
    Exp = mybir.ActivationFunctionType.Exp

    singles = ctx.enter_context(tc.tile_pool(name="singles", bufs=1))
    woA_sb = singles.tile([128, C], bf16)          # Wo rows for heads 0,1
    woB_sb = singles.tile([64, C], bf16)           # Wo rows for head 2
    lams_sb = singles.tile([128, 6], fp32)         # col u: 1.0 (br0) / -lam (br1)
    xt_sb = singles.tile([128, 6, N], bf16)        # x^T, c = ch*128+p
    wqk_sb = singles.tile([128, 6, C], bf16)       # blocks [q0 k0 q1 k1 q2 k2]
    wv_sb = singles.tile([128, 6, HPC * D], bf16)
    qk_sb = singles.tile([128, HPC, 2, N], bf16)   # [part(br*64+d), h, q/k, n]
    v_sb = singles.tile([128, NT, HPC, D + 1], bf16)
    u_sb = singles.tile([65, HPC, 2, N], fp32)     # rows 0:64 u, row 64 denom
    ddA = singles.tile([128, N], bf16)             # diff for heads 0,1
    ddB = singles.tile([64, N], bf16)              # diff for head 2
    r_dram = nc.dram_tensor("r_bounce", [6, N], bf16)

    nc.sync.dma_start(out=woA_sb, in_=wo[0:128, :])
    nc.sync.dma_start(out=woB_sb, in_=wo[128:192, :])
    nc.sync.dma_start(out=lams_sb, in_=lamc[:, :])
    xt_r = xt[:, :].rearrange("(ch p) n -> p ch n", p=128)
    wqk_r = wqk[:, :].rearrange("(ch p) w -> p ch w", p=128)
    wv_r = wv[:, :].rearrange("(ch p) w -> p ch w", p=128)
    for c in range(6):
        nc.sync.dma_start(out=wv_sb[:, c, :], in_=wv_r[:, c, :])
    for c in range(6):
        eng = nc.sync if c % 2 == 0 else nc.gpsimd
        eng.dma_start(out=xt_sb[:, c, :], in_=xt_r[:, c, :])
    for c in range(6):
        nc.sync.dma_start(out=wqk_sb[:, c, :], in_=wqk_r[:, c, :])
    nc.vector.memset(v_sb[:, :, :, D : D + 1], 1.0)

    # pre-warm PE clock gate + preload the exp table during the input DMA wait
    with tc.tile_pool(name="warm_sb", bufs=1) as warm_sb, \
         tc.tile_pool(name="warm_ps", bufs=1, space="PSUM") as warm_ps:
        wsrc = warm_sb.tile([128, 512], bf16)
        nc.vector.memset(wsrc, 0.0)
        wdst = warm_sb.tile([128, 16], bf16)
        wt = warm_ps.tile([128, 512], fp32)
        nc.scalar.activation(wdst, wsrc[:, 0:16], Exp)
        for _ in range(20):
            nc.tensor.matmul(wt, lhsT=wsrc[:, 0:128], rhs=wsrc, start=True, stop=True)

    # ---------- projections (own PSUM scope; must fully precede attention:
    # proj matmuls interleaved with the row-tiled scores come back corrupt) --
    with tc.tile_pool(name="pjp", bufs=2, space="PSUM") as pjp:
        # v projection, all heads, contraction-split T0/T8 like q/k (one
        # accumulation group per psum bank: start=True clears its bank)
        # DVE cannot read two PSUM operands in one op (NCC_IBVF027): the idle
        # ScalarE copies the T8 partial to SBUF, DVE adds PSUM + SBUF
        with tc.tile_pool(name="pjt", bufs=2) as pjt:
            for ti in range(NT):
                vpA = pjp.tile([128, HPC * D], fp32, tag="vA", name="vpA")
                vpB = pjp.tile([128, HPC * D], fp32, tag="vB", name="vpB")
                for c in range(6):
                    for p0, vp in ((0, vpA), (64, vpB)):
                        nc.tensor.matmul(
                            vp,
                            lhsT=xt_sb[p0 : p0 + 64, c, ti * 128 : (ti + 1) * 128],
                            rhs=wv_sb[p0 : p0 + 64, c, :],
                            start=(c == 0),
                            stop=(c == 5),
                            tile_position=(p0, 0),
                        )
                tvB = pjt.tile([128, HPC * D], bf16, tag="tvB", name="tvB")
                nc.scalar.copy(tvB, vpB)
                nc.vector.tensor_add(
                    v_sb[:, ti, :, 0:D],
                    vpA.rearrange("p (h d) -> p h d", h=HPC),
                    tvB.rearrange("p (h d) -> p h d", h=HPC),
                )
            for hh in range(HPC):
                _proj_head(nc, pjp, pjt, xt_sb, wqk_sb, qk_sb, hh)

    # EXP group pattern per (h, qi): 2-strip "big" tiles alternating with
    # 1-strip "small" ones -> ACT calls of 2048/1024 elems, amortizing the
    # ~280-cycle per-instruction overhead while fitting 8 PSUM banks
    GROUPS = [(0, 1), (2,), (3, 4), (5,), (6, 7), (8,),
              (9, 10), (11,), (12, 13), (14,), (15,)]

    with tc.tile_pool(name="stB", bufs=1, space="PSUM") as stB, \
         tc.tile_pool(name="stS", bufs=1, space="PSUM") as stS, \
         tc.tile_pool(name="upp", bufs=1, space="PSUM") as upp, \
         tc.tile_pool(name="ptp", bufs=2) as ptp, \
         tc.tile_pool(name="rsc", bufs=3) as rsc:

        dens = {}

        def emit_rchain(h, qi):
            """reciprocal of the spread denominators for one (h, qi) slot;
            result lands in r_dram (bf16, -lam folded for branch 1)."""
            for br in range(2):
                u = 2 * h + br
                rq = rsc.tile([128, 4], fp32, tag=f"rq{br}", name=f"rq{br}")
                nc.vector.reciprocal(rq, dens[h][br][:, qi * 4 : qi * 4 + 4])
                rbq = rsc.tile([128, 4], bf16, tag=f"rBq{br}", name=f"rBq{br}")
                nc.vector.tensor_scalar_mul(rbq, rq, lams_sb[:, u : u + 1])
                nc.sync.dma_start(
                    out=r_dram[u : u + 1, qi * QB : (qi + 1) * QB], in_=rbq
                )

        def emit_diff(h, qi):
            """diff_h[:, qi] = u0*r0 - lam*u1*r1 (lam folded into r1)."""
            q0 = qi * QB
            rb = []
            for br in range(2):
                u = 2 * h + br
                rbx = rsc.tile([64, QB], bf16, tag=f"rb{br}", name=f"rb{br}")
                nc.sync.dma_start(
                    out=rbx,
                    in_=r_dram[u : u + 1, q0 : q0 + QB].partition_broadcast(64),
                )
                rb.append(rbx)
            t1 = rsc.tile([64, QB], bf16, tag="t1", name="t1")
            nc.vector.tensor_mul(t1, u_sb[0:64, h, 0, q0 : q0 + QB], rb[0])
            t2 = rsc.tile([64, QB], bf16, tag="t2", name="t2")
            nc.vector.tensor_mul(t2, u_sb[0:64, h, 1, q0 : q0 + QB], rb[1])
            dst = (ddA[h * 64 : (h + 1) * 64, q0 : q0 + QB] if h < 2
                   else ddB[:, q0 : q0 + QB])
            nc.vector.tensor_add(dst, t1, t2)

        # ---------- attention ----------
        slots = [(h, qi) for h in range(HPC) for qi in range(NQ)]
        for si, (h, qi) in enumerate(slots):
            if qi == 0:
                dens[h] = [
                    rsc.tile([128, NQ * 4], fp32, tag=f"den{br}", name=f"den{br}")
                    for br in range(2)
                ]
            q0 = qi * QB
            u_pair = [
                upp.tile([65, QB], fp32, tag=f"u{br}", name=f"u_ps{br}")
                for br in range(2)
            ]
            def emit_pv(entry):
                for ptt, j, s in entry:
                    ptj = ptt[:, j, :, :] if j is not None else ptt
                    for br in range(2):
                        nc.tensor.matmul(
                            u_pair[br],
                            lhsT=v_sb[:, s, h, :],
                            rhs=ptj[:, br, :],
                            start=(s == 0),
                            stop=(s == NT - 1),
                        )

            # PV emission lags the exp by TWO groups: the PE FIFO then runs
            # the next scores the moment their st bank frees, instead of
            # first draining a PV burst that is gated on the current exp —
            # keeps the ACT stream gap-free
            pending = []
            for g in GROUPS:
                if len(g) == 2:
                    st = stB.tile([128, 2, 2, QB], fp32, tag="stB", name="stb")
                    pt = ptp.tile([128, 2, 2, QB], bf16, tag="ptB", name="ptb")
                else:
                    st = stS.tile([128, 2, QB], fp32, tag="stS", name="sts")
                    pt = ptp.tile([128, 2, QB], bf16, tag="ptS", name="pts")
                for j, s in enumerate(g):
                    stj = st[:, j, :, :] if len(g) == 2 else st
                    for br in range(2):
                        p0 = br * 64
                        nc.tensor.matmul(
                            stj[:, br, :],
                            lhsT=qk_sb[p0 : p0 + 64, h, 1, s * 128 : (s + 1) * 128],
                            rhs=qk_sb[p0 : p0 + 64, h, 0, q0 : q0 + QB],
                            start=True,
                            stop=True,
                            tile_position=(p0, 0),
                        )
                nc.scalar.activation(pt, st, Exp)
                pending.append([(pt, (j if len(g) == 2 else None), s)
                                for j, s in enumerate(g)])
                if len(pending) > 2:
                    emit_pv(pending.pop(0))
            for entry in pending:
                emit_pv(entry)
            for br in range(2):
                nc.vector.tensor_copy(u_sb[:, h, br, q0 : q0 + QB], u_pair[br])
                # spread denom [1,QB] -> [128, QB/128] for full-lane recip
                nc.sync.dma_start(
                    out=dens[h][br][:, qi * 4 : qi * 4 + 4],
                    in_=u_sb[64:65, h, br, q0 : q0 + QB],
                )
            # lagged post-processing so DMA latency hides under attention:
            # recip chain one slot behind, diff two slots behind
            if si >= 1:
                emit_rchain(*slots[si - 1])
            if si >= 2:
                emit_diff(*slots[si - 2])
        emit_rchain(*slots[-1])
        # PE keep-warm: gated on the last u evacuation via their operands,
        # these cheap junk matmuls (fp32, free=64) keep the HAM clock at
        # 2.4 GHz through the final recip/diff DMA chain so the output
        # projection is not throttled to 1.2 GHz
        junk = stS.tile([128, 2, QB], fp32, tag="stS", name="junk")
        for s in range(40):
            off = (NQ - 1) * QB + (s % 4) * 128
            nc.tensor.matmul(
                junk[:, 0, 0:64],
                lhsT=u_sb[0:64, HPC - 1, 1, off : off + 128],
                rhs=u_sb[0:64, HPC - 1, 1, off : off + 64],
                start=True,
                stop=True,
            )
        emit_diff(*slots[-2])
        emit_diff(*slots[-1])
        if taps:
            nc.sync.dma_start(out=taps["qk"][:, :, :, :], in_=qk_sb)
            nc.sync.dma_start(out=taps["v"][:, :, :, :], in_=v_sb)
            nc.sync.dma_start(out=taps["u"][:, :, :, :], in_=u_sb)
            nc.sync.dma_start(out=taps["r"][0, :, :], in_=r_dram[:, :])
            nc.sync.dma_start(out=taps["ddA"][:, :], in_=ddA)
            nc.sync.dma_start(out=taps["ddB"][:, :], in_=ddB)

    # ---------- output projection: contraction 192 = 128 (h0,h1) + 64 (h2) ----
    with tc.tile_pool(name="fpp", bufs=3, space="PSUM") as fpp, \
         tc.tile_pool(name="outp", bufs=3) as outp:
        for ti in range(NT):
            fo = fpp.tile([128, C], fp32)
            for o, w in ((0, 512), (512, 256)):
                nc.tensor.matmul(
                    fo[:, o : o + w],
                    lhsT=ddA[:, ti * 128 : (ti + 1) * 128],
                    rhs=woA_sb[:, o : o + w],
                    start=True,
                    stop=False,
                )
                nc.tensor.matmul(
                    fo[:, o : o + w],
                    lhsT=ddB[:, ti * 128 : (ti + 1) * 128],
                    rhs=woB_sb[:, o : o + w],
                    start=False,
                    stop=True,
                )
            ot = outp.tile([128, C], bf16)
            if ti % 2 == 0:
                nc.vector.tensor_copy(ot, fo)
            else:
                nc.scalar.copy(ot, fo)   # ACT is idle here; split the copies
            oeng = nc.sync if ti % 2 == 0 else nc.gpsimd
            oeng.dma_start(out=out[ti * 128 : (ti + 1) * 128, :], in_=ot)


def build_bass(debug_taps=False):
    nc = bacc_mod.Bacc(None)
    xt = nc.dram_tensor("xt", [C, N], F.bfloat16, kind="ExternalInput")
    wqk = nc.dram_tensor("wqk", [C, C], F.bfloat16, kind="ExternalInput")
    wv = nc.dram_tensor("wv", [C, HPC * D], F.bfloat16, kind="ExternalInput")
    wo = nc.dram_tensor("wo", [HPC * D, C], F.bfloat16, kind="ExternalInput")
    lamc = nc.dram_tensor("lamc", [128, 6], F.float32, kind="ExternalInput")
    out = nc.dram_tensor("out", [N, C], F.bfloat16, kind="ExternalOutput")
    taps = None
    if debug_taps:
        taps = {
            "qk": nc.dram_tensor("tap_qk", [128, HPC, 2, N], F.bfloat16, kind="ExternalOutput"),
            "v": nc.dram_tensor("tap_v", [128, NT, HPC, D + 1], F.bfloat16, kind="ExternalOutput"),
            "u": nc.dram_tensor("tap_u", [65, HPC, 2, N], F.float32, kind="ExternalOutput"),
            "r": nc.dram_tensor("tap_r", [1, 6, N], F.bfloat16, kind="ExternalOutput"),
            "ddA": nc.dram_tensor("tap_ddA", [128, N], F.bfloat16, kind="ExternalOutput"),
            "ddB": nc.dram_tensor("tap_ddB", [64, N], F.bfloat16, kind="ExternalOutput"),
        }
    with TileContext(nc) as tc:
        with ExitStack() as ctx:
            _body(nc, tc, ctx, xt, wqk, wv, wo, lamc, out, taps=taps)
    nc.compile()
    return nc


_NC = None


def _get_nc():
    global _NC
    if _NC is None:
        _NC = build_bass()
    return _NC


def _prep_core(core, x, Wq, Wk, Wv, Wo, lam):
    b = core // 4
    heads = [(core % 4) * HPC + i for i in range(HPC)]
    sc = 1.0 / np.sqrt(D)
    xt = np.ascontiguousarray(x[b].T).astype(BF16)
    # block layout [q_h0 | k_h0 | q_h1 | k_h1 | q_h2 | k_h2], within each
    # 128-col block: cols 0:64 = branch0, cols 64:128 = branch1
    wqk = np.empty((C, C), np.float32)
    for i, h in enumerate(heads):
        qb, kb = (2 * i) * 128, (2 * i + 1) * 128
        for br in range(2):
            wqk[:, qb + br * 64 : qb + br * 64 + 64] = (
                Wq[:, br * C + h * D : br * C + (h + 1) * D] * sc
            )
            wqk[:, kb + br * 64 : kb + br * 64 + 64] = (
                Wk[:, br * C + h * D : br * C + (h + 1) * D]
            )
    wv = np.concatenate([Wv[:, h * D : (h + 1) * D] for h in heads], axis=1)
    wo = np.concatenate([Wo[h * D : (h + 1) * D, :] for h in heads], axis=0)
    lams = np.zeros((128, 6), np.float32)
    for i, h in enumerate(heads):
        lams[:, 2 * i] = 1.0
        lams[:, 2 * i + 1] = -lam[h]
    return dict(
        xt=xt,
        wqk=wqk.astype(BF16),
        wv=wv.astype(BF16),
        wo=wo.astype(BF16),
        lamc=lams,
    )


def kernel(x, Wq, Wk, Wv, lambda_p, Wo, bo, _trace=False, _tmpdir=None):
    x = np.asarray(x, np.float32)
    lam = np.exp(np.asarray(lambda_p, np.float32).reshape(H))
    in_maps = [
        _prep_core(core, x, np.asarray(Wq, np.float32), np.asarray(Wk, np.float32),
                   np.asarray(Wv, np.float32), np.asarray(Wo, np.float32), lam)
        for core in range(NCORES)
    ]
    nc = _get_nc()
    res = run_bass_kernel_spmd(
        nc, in_maps, list(range(NCORES)), trace=_trace, tmpdir=_tmpdir
    )
    outf = np.zeros((B, N, C), np.float32)
    for core in range(NCORES):
        outf[core // 4] += np.asarray(res.results[core]["out"], np.float32)
    outf += np.asarray(bo, np.float32)[None, None, :]
    if _trace:
        kernel.last_exec_time_ns = res.exec_time_ns
    return outf


# revision 21
# speedup vs baseline: 1.1894x; 1.0972x over previous
"""Differential attention kernel for Trainium2, 8 NeuronCores.

Sharding: B(2) x head-groups(4) -> 8 cores; each core computes 3 heads'
differential attention for one batch element plus its partial slice of the
output projection (row-parallel over Wo). Host sums the 4 partials per batch
element and adds bo.

v2 pipeline (per core, all matmuls bf16, fp32 PSUM accum):
  - scores for the two branches run CONCURRENTLY in the PE via row tiling
    (tile_position (0,0) and (64,0)): branch-b q^T/k^T live on SBUF
    partitions b*64..b*64+64, each score matmul contracts over its 64-row
    group, both stream at once -> 2x score throughput.
  - exp batched: ONE ScalarE activation per (head, q-block, strip) covering
    both branches' S^T tiles ([128, 2, 512]) -> fewer, larger ACT calls.
  - PV keeps the [v|1] M=65 trick (denominator rides as psum row 64),
    software-pipelined one strip behind exp.
  - next head's q/k projections are drip-fed into the attention strip loop
    so the PE's slack under the ACT-bound steady state does the projections
    for free; ACT starts on head 0 ~40us earlier than a serial-proj design.
  - output projection contracts 192 rows as 128+64 packed matmuls.
"""

import os
import sys
from contextlib import ExitStack

for _p in ("/opt/trn_rl_repo", "/root/.axon_site/_ro/trn_rl_repo"):
    if os.path.isdir(_p) and _p not in sys.path:
        sys.path.insert(0, _p)

import ml_dtypes
import numpy as np

import concourse.bass as bass
import concourse.bacc as bacc_mod
import concourse.mybir as mybir
from concourse.bass_utils import run_bass_kernel_spmd
from concourse.tile import TileContext

BF16 = ml_dtypes.bfloat16
F = mybir.dt

B, N, C, H, D = 2, 2048, 768, 12, 64
HPC = 3          # heads per core
NCORES = 8
NT = N // 128    # 16 key strips
QB = 512         # query-block width (one PSUM bank of fp32)
NQ = N // QB     # 4 query blocks



def _proj_head(nc, pjp, xt_sb, wqk_sb, qk_sb, h):
    """q/k projection for head h: 8 groups (q,k x 4 query-quarters), each
    6 accumulating matmuls (full 128-contraction) + 1 psum->SBUF copy.
    (Contraction-splitting onto T0/T8 row-groups is a wash: it doubles the
    streamed columns and the concurrency only wins that factor back.)"""
    fp32 = F.float32
    for t in range(2):          # 0 = q, 1 = k
        blk = (2 * h + t) * 128
        for g in range(4):
            pp = pjp.tile([128, 512], fp32, tag="pjA", name="pp")
            for c in range(6):
                nc.tensor.matmul(
                    pp,
                    lhsT=wqk_sb[:, c, blk : blk + 128],
                    rhs=xt_sb[:, c, g * 512 : (g + 1) * 512],
                    start=(c == 0),
                    stop=(c == 5),
                )
            nc.vector.tensor_copy(qk_sb[:, h, t, g * 512 : (g + 1) * 512], pp)


def _body(nc, tc, ctx, xt, wqk, wv, wo, lamc, out, taps=None):
    fp32, bf16 = F.float32, F.bfloat16
    Exp = mybir.ActivationFunctionType.Exp

    singles = ctx.enter_context(tc.tile_pool(name="singles", bufs=1))
    woA_sb = singles.tile([128, C], bf16)          # Wo rows for heads 0,1
    woB_sb = singles.tile([64, C], bf16)           # Wo rows for head 2
    lams_sb = singles.tile([128, 6], fp32)         # col u: 1.0 (br0) / -lam (br1)
    xt_sb = singles.tile([128, 6, N], bf16)        # x^T, c = ch*128+p
    wqk_sb = singles.tile([128, 6, C], bf16)       # blocks [q0 k0 q1 k1 q2 k2]
    wv_sb = singles.tile([128, 6, HPC * D], bf16)
    qk_sb = singles.tile([128, HPC, 2, N], bf16)   # [part(br*64+d), h, q/k, n]
    v_sb = singles.tile([128, NT, HPC, D + 1], bf16)
    u_sb = singles.tile([65, HPC, 2, N], fp32)     # rows 0:64 u, row 64 denom
    ddA = singles.tile([128, N], bf16)             # diff for heads 0,1
    ddB = singles.tile([64, N], bf16)              # diff for head 2
    r_dram = nc.dram_tensor("r_bounce", [6, N], bf16)

    nc.sync.dma_start(out=woA_sb, in_=wo[0:128, :])
    nc.sync.dma_start(out=woB_sb, in_=wo[128:192, :])
    nc.sync.dma_start(out=lams_sb, in_=lamc[:, :])
    xt_r = xt[:, :].rearrange("(ch p) n -> p ch n", p=128)
    wqk_r = wqk[:, :].rearrange("(ch p) w -> p ch w", p=128)
    wv_r = wv[:, :].rearrange("(ch p) w -> p ch w", p=128)
    for c in range(6):
        nc.sync.dma_start(out=wv_sb[:, c, :], in_=wv_r[:, c, :])
    for c in range(6):
        eng = nc.sync if c % 2 == 0 else nc.gpsimd
        eng.dma_start(out=xt_sb[:, c, :], in_=xt_r[:, c, :])
    for c in range(6):
        nc.sync.dma_start(out=wqk_sb[:, c, :], in_=wqk_r[:, c, :])
    nc.vector.memset(v_sb[:, :, :, D : D + 1], 1.0)

    # pre-warm PE clock gate + preload the exp table during the input DMA wait
    with tc.tile_pool(name="warm_sb", bufs=1) as warm_sb, \
         tc.tile_pool(name="warm_ps", bufs=1, space="PSUM") as warm_ps:
        wsrc = warm_sb.tile([128, 512], bf16)
        nc.vector.memset(wsrc, 0.0)
        wdst = warm_sb.tile([128, 16], bf16)
        wt = warm_ps.tile([128, 512], fp32)
        nc.scalar.activation(wdst, wsrc[:, 0:16], Exp)
        for _ in range(20):
            nc.tensor.matmul(wt, lhsT=wsrc[:, 0:128], rhs=wsrc, start=True, stop=True)

    # ---------- projections (own PSUM scope; must fully precede attention:
    # proj matmuls interleaved with the row-tiled scores come back corrupt) --
    with tc.tile_pool(name="pjp", bufs=2, space="PSUM") as pjp:
        # v projection, all heads, contraction-split T0/T8 like q/k (one
        # accumulation group per psum bank: start=True clears its bank)
        for ti in range(NT):
            vp = pjp.tile([128, HPC * D], fp32, tag="vA", name="vp")
            for c in range(6):
                nc.tensor.matmul(
                    vp,
                    lhsT=xt_sb[:, c, ti * 128 : (ti + 1) * 128],
                    rhs=wv_sb[:, c, :],
                    start=(c == 0),
                    stop=(c == 5),
                )
            nc.vector.tensor_copy(
                v_sb[:, ti, :, 0:D], vp.rearrange("p (h d) -> p h d", h=HPC)
            )
        for hh in range(HPC):
            _proj_head(nc, pjp, xt_sb, wqk_sb, qk_sb, hh)

    # EXP group pattern per (h, qi): 2-strip "big" tiles alternating with
    # 1-strip "small" ones -> ACT calls of 2048/1024 elems, amortizing the
    # ~280-cycle per-instruction overhead while fitting 8 PSUM banks
    GROUPS = [(0, 1), (2,), (3, 4), (5,), (6, 7), (8,),
              (9, 10), (11,), (12, 13), (14,), (15,)]

    with tc.tile_pool(name="stB", bufs=1, space="PSUM") as stB, \
         tc.tile_pool(name="stS", bufs=1, space="PSUM") as stS, \
         tc.tile_pool(name="upp", bufs=1, space="PSUM") as upp, \
         tc.tile_pool(name="ptp", bufs=2) as ptp, \
         tc.tile_pool(name="rsc", bufs=3) as rsc:

        dens = {}

        def emit_rchain(h, qi):
            """reciprocal of the spread denominators for one (h, qi) slot;
            result lands in r_dram (bf16, -lam folded for branch 1)."""
            for br in range(2):
                u = 2 * h + br
                rq = rsc.tile([128, 4], fp32, tag=f"rq{br}", name=f"rq{br}")
                nc.vector.reciprocal(rq, dens[h][br][:, qi * 4 : qi * 4 + 4])
                rbq = rsc.tile([128, 4], bf16, tag=f"rBq{br}", name=f"rBq{br}")
                nc.vector.tensor_scalar_mul(rbq, rq, lams_sb[:, u : u + 1])
                nc.sync.dma_start(
                    out=r_dram[u : u + 1, qi * QB : (qi + 1) * QB], in_=rbq
                )

        def emit_diff(h, qi):
            """diff_h[:, qi] = u0*r0 - lam*u1*r1 (lam folded into r1)."""
            q0 = qi * QB
            rb = []
            for br in range(2):
                u = 2 * h + br
                rbx = rsc.tile([64, QB], bf16, tag=f"rb{br}", name=f"rb{br}")
                nc.sync.dma_start(
                    out=rbx,
                    in_=r_dram[u : u + 1, q0 : q0 + QB].partition_broadcast(64),
                )
                rb.append(rbx)
            t1 = rsc.tile([64, QB], bf16, tag="t1", name="t1")
            nc.vector.tensor_mul(t1, u_sb[0:64, h, 0, q0 : q0 + QB], rb[0])
            t2 = rsc.tile([64, QB], bf16, tag="t2", name="t2")
            nc.vector.tensor_mul(t2, u_sb[0:64, h, 1, q0 : q0 + QB], rb[1])
            dst = (ddA[h * 64 : (h + 1) * 64, q0 : q0 + QB] if h < 2
                   else ddB[:, q0 : q0 + QB])
            nc.vector.tensor_add(dst, t1, t2)

        # ---------- attention ----------
        # PV emission lags the exp by TWO groups (the PE FIFO then runs the
        # next scores the moment their st bank frees instead of draining a
        # PV burst gated on the current exp), and the pending queue carries
        # ACROSS qi/head boundaries so the ACT stream never sees a drain
        # bubble. Slot post-processing (u evac, denom spread, recip chain,
        # diff) rides on the pop of its slot's last PV group.
        slots = [(h, qi) for h in range(HPC) for qi in range(NQ)]
        pending = []    # [(pv_fn, end_fn_or_None)]

        def flush(keep):
            while len(pending) > keep:
                pv_fn, end_fn = pending.pop(0)
                pv_fn()
                if end_fn is not None:
                    end_fn()

        def make_pv(entry, u_pair, h):
            def pv_fn():
                for ptt, j, s in entry:
                    ptj = ptt[:, j, :, :] if j is not None else ptt
                    for br in range(2):
                        nc.tensor.matmul(
                            u_pair[br],
                            lhsT=v_sb[:, s, h, :],
                            rhs=ptj[:, br, :],
                            start=(s == 0),
                            stop=(s == NT - 1),
                        )
            return pv_fn

        def make_end(si, u_pair):
            def end_fn():
                h, qi = slots[si]
                q0 = qi * QB
                for br in range(2):
                    nc.vector.tensor_copy(
                        u_sb[:, h, br, q0 : q0 + QB], u_pair[br]
                    )
                    # spread denom [1,QB] -> [128,4] for full-lane recip
                    nc.sync.dma_start(
                        out=dens[h][br][:, qi * 4 : qi * 4 + 4],
                        in_=u_sb[64:65, h, br, q0 : q0 + QB],
                    )
                emit_rchain(h, qi)
                if si >= 1:
                    emit_diff(*slots[si - 1])
            return end_fn

        for si, (h, qi) in enumerate(slots):
            if qi == 0:
                dens[h] = [
                    rsc.tile([128, NQ * 4], fp32, tag=f"den{br}", name=f"den{br}")
                    for br in range(2)
                ]
            q0 = qi * QB
            u_pair = [
                upp.tile([65, QB], fp32, tag=f"u{br}", name=f"u_ps{br}")
                for br in range(2)
            ]
            for gi, g in enumerate(GROUPS):
                if len(g) == 2:
                    st = stB.tile([128, 2, 2, QB], fp32, tag="stB", name="stb")
                    pt = ptp.tile([128, 2, 2, QB], bf16, tag="ptB", name="ptb")
                else:
                    st = stS.tile([128, 2, QB], fp32, tag="stS", name="sts")
                    pt = ptp.tile([128, 2, QB], bf16, tag="ptS", name="pts")
                for j, s in enumerate(g):
                    stj = st[:, j, :, :] if len(g) == 2 else st
                    for br in range(2):
                        p0 = br * 64
                        nc.tensor.matmul(
                            stj[:, br, :],
                            lhsT=qk_sb[p0 : p0 + 64, h, 1, s * 128 : (s + 1) * 128],
                            rhs=qk_sb[p0 : p0 + 64, h, 0, q0 : q0 + QB],
                            start=True,
                            stop=True,
                            tile_position=(p0, 0),
                        )
                nc.scalar.activation(pt, st, Exp)
                entry = [(pt, (j if len(g) == 2 else None), s)
                         for j, s in enumerate(g)]
                last = gi == len(GROUPS) - 1
                pending.append((make_pv(entry, u_pair, h),
                                make_end(si, u_pair) if last else None))
                flush(2)
        flush(0)
        emit_diff(*slots[-1])
        if taps:
            nc.sync.dma_start(out=taps["qk"][:, :, :, :], in_=qk_sb)
            nc.sync.dma_start(out=taps["v"][:, :, :, :], in_=v_sb)
            nc.sync.dma_start(out=taps["u"][:, :, :, :], in_=u_sb)
            nc.sync.dma_start(out=taps["r"][0, :, :], in_=r_dram[:, :])
            nc.sync.dma_start(out=taps["ddA"][:, :], in_=ddA)
            nc.sync.dma_start(out=taps["ddB"][:, :], in_=ddB)

    # ---------- output projection: contraction 192 = 128 (h0,h1) + 64 (h2) ----
    with tc.tile_pool(name="fpp", bufs=3, space="PSUM") as fpp, \
         tc.tile_pool(name="outp", bufs=3) as outp:
        for ti in range(NT):
            fo = fpp.tile([128, C], fp32)
            for o, w in ((0, 512), (512, 256)):
                nc.tensor.matmul(
                    fo[:, o : o + w],
                    lhsT=ddA[:, ti * 128 : (ti + 1) * 128],
                    rhs=woA_sb[:, o : o + w],
                    start=True,
                    stop=False,
                )
                nc.tensor.matmul(
                    fo[:, o : o + w],
                    lhsT=ddB[:, ti * 128 : (ti + 1) * 128],
                    rhs=woB_sb[:, o : o + w],
                    start=False,
                    stop=True,
                )
            ot = outp.tile([128, C], bf16)
            if ti % 2 == 0:
                nc.vector.tensor_copy(ot, fo)
            else:
                nc.scalar.copy(ot, fo)   # ACT is idle here; split the copies
            oeng = nc.sync if ti % 2 == 0 else nc.gpsimd
            oeng.dma_start(out=out[ti * 128 : (ti + 1) * 128, :], in_=ot)


def build_bass(debug_taps=False):
    nc = bacc_mod.Bacc(None)
    xt = nc.dram_tensor("xt", [C, N], F.bfloat16, kind="ExternalInput")
    wqk = nc.dram_tensor("wqk", [C, C], F.bfloat16, kind="ExternalInput")
    wv = nc.dram_tensor("wv", [C, HPC * D], F.bfloat16, kind="ExternalInput")
    wo = nc.dram_tensor("wo", [HPC * D, C], F.bfloat16, kind="ExternalInput")
    lamc = nc.dram_tensor("lamc", [128, 6], F.float32, kind="ExternalInput")
    out = nc.dram_tensor("out", [N, C], F.bfloat16, kind="ExternalOutput")
    taps = None
    if debug_taps:
        taps = {
            "qk": nc.dram_tensor("tap_qk", [128, HPC, 2, N], F.bfloat16, kind="ExternalOutput"),
            "v": nc.dram_tensor("tap_v", [128, NT, HPC, D + 1], F.bfloat16, kind="ExternalOutput"),
            "u": nc.dram_tensor("tap_u", [65, HPC, 2, N], F.float32, kind="ExternalOutput"),
            "r": nc.dram_tensor("tap_r", [1, 6, N], F.bfloat16, kind="ExternalOutput"),
            "ddA": nc.dram_tensor("tap_ddA", [128, N], F.bfloat16, kind="ExternalOutput"),
            "ddB": nc.dram_tensor("tap_ddB", [64, N], F.bfloat16, kind="ExternalOutput"),
        }
    with TileContext(nc) as tc:
        with ExitStack() as ctx:
            _body(nc, tc, ctx, xt, wqk, wv, wo, lamc, out, taps=taps)
    nc.compile()
    return nc


_NC = None


def _get_nc():
    global _NC
    if _NC is None:
        _NC = build_bass()
    return _NC


def _prep_core(core, x, Wq, Wk, Wv, Wo, lam):
    b = core // 4
    heads = [(core % 4) * HPC + i for i in range(HPC)]
    sc = 1.0 / np.sqrt(D)
    xt = np.ascontiguousarray(x[b].T).astype(BF16)
    # block layout [q_h0 | k_h0 | q_h1 | k_h1 | q_h2 | k_h2], within each
    # 128-col block: cols 0:64 = branch0, cols 64:128 = branch1
    wqk = np.empty((C, C), np.float32)
    for i, h in enumerate(heads):
        qb, kb = (2 * i) * 128, (2 * i + 1) * 128
        for br in range(2):
            wqk[:, qb + br * 64 : qb + br * 64 + 64] = (
                Wq[:, br * C + h * D : br * C + (h + 1) * D] * sc
            )
            wqk[:, kb + br * 64 : kb + br * 64 + 64] = (
                Wk[:, br * C + h * D : br * C + (h + 1) * D]
            )
    wv = np.concatenate([Wv[:, h * D : (h + 1) * D] for h in heads], axis=1)
    wo = np.concatenate([Wo[h * D : (h + 1) * D, :] for h in heads], axis=0)
    lams = np.zeros((128, 6), np.float32)
    for i, h in enumerate(heads):
        lams[:, 2 * i] = 1.0
        lams[:, 2 * i + 1] = -lam[h]
    return dict(
        xt=xt,
        wqk=wqk.astype(BF16),
        wv=wv.astype(BF16),
        wo=wo.astype(BF16),
        lamc=lams,
    )


def kernel(x, Wq, Wk, Wv, lambda_p, Wo, bo, _trace=False, _tmpdir=None):
    x = np.asarray(x, np.float32)
    lam = np.exp(np.asarray(lambda_p, np.float32).reshape(H))
    in_maps = [
        _prep_core(core, x, np.asarray(Wq, np.float32), np.asarray(Wk, np.float32),
                   np.asarray(Wv, np.float32), np.asarray(Wo, np.float32), lam)
        for core in range(NCORES)
    ]
    nc = _get_nc()
    res = run_bass_kernel_spmd(
        nc, in_maps, list(range(NCORES)), trace=_trace, tmpdir=_tmpdir
    )
    outf = np.zeros((B, N, C), np.float32)
    for core in range(NCORES):
        outf[core // 4] += np.asarray(res.results[core]["out"], np.float32)
    outf += np.asarray(bo, np.float32)[None, None, :]
    if _trace:
        kernel.last_exec_time_ns = res.exec_time_ns
    return outf
